# revision 17
# baseline (speedup 1.0000x reference)
"""Block-causal segmented attention on 8 TRN2 NeuronCores.

Sharding: DP over batch (cores 0-3 -> b=0, cores 4-7 -> b=1) x TP over heads
(each core owns 2 of 8 heads).  Everything runs in transposed (feature-major)
layout so no on-device transposes are needed:
  - projections compute qT/kT (head_dim x tokens) directly; v as (tokens x d)
  - scoresT = kT_tile.T @ qT  (keys on partitions, queries on free axis)
  - masking (segment + block-causal + attention_mask + tile edges) is applied
    as a per-partition (per-key) bias column inside the exp activation
  - avT = v_tile.T @ expT accumulates in PSUM; row-sums via ones-column matmul
  - one AllToAll turns head-sharded avT into token-sharded full-feature attnT
  - out_chunk = attnT.T @ woT per core; host concatenates the 8 shards.
RoPE: even/odd d-permutation is folded into the weight shards on the host, so
the on-device rotation is elementwise muls with host-built cos/sin tables plus
a partition-half swap done with two SBUF->SBUF DMAs.
"""

import os
import sys

for _p in ("/opt/trn_rl_repo",):
    if _p not in sys.path:
        sys.path.insert(0, _p)

import numpy as np
import ml_dtypes

import concourse.bass as bass
import concourse.bacc as bacc
import concourse.mybir as mybir
from concourse import tile
from concourse.bass_utils import run_bass_kernel_spmd

BF16 = ml_dtypes.bfloat16
F32 = mybir.dt.float32
BF = mybir.dt.bfloat16

HIDDEN = 1024
HEADS = 8
HD = 128
B = 2
L = 4096
MAX_TOKS = 2048
THETA = 10000.0
NCORES = 8
TP = 4          # cores per batch element
HPC = 2         # heads per core
FS = HPC * HD   # 256 features per core
TQ = L // TP    # 1024 tokens per core for the output projection
SCALE = 1.0 / float(np.sqrt(HD))
NEG = -30.0

LAST_RESULTS = None

ts = bass.ts


def _plan_chunks(seg_starts):
    """Shared (SPMD-uniform) q-chunk structure.

    Chunk boundaries = union of both batches' segment starts + the 2048 block
    boundary, then split to <=512.  Per chunk, per batch: allowed key interval
    [lo_b, hi_b); the compile-time key-tile range is the union over b.
    """
    bounds = sorted(set([0, L, MAX_TOKS] + [s for s in seg_starts if 0 < s < L]))
    chunks = []
    col = 0
    for a, bnd in zip(bounds[:-1], bounds[1:]):
        q0 = a
        while q0 < bnd:
            qlen = min(512, bnd - q0)
            per_b = []
            for bb in range(B):
                s = seg_starts[bb]
                seg_lo, seg_hi = (0, s) if q0 < s else (s, L)
                blk_end = MAX_TOKS if q0 < MAX_TOKS else L
                per_b.append((seg_lo, min(seg_hi, blk_end)))
            lo_u = min(lo for lo, hi in per_b)
            hi_u = max(hi for lo, hi in per_b)
            ktlo, kthi = lo_u // HD, (hi_u + HD - 1) // HD
            chunks.append(dict(q0=q0, qlen=qlen, per_b=per_b,
                               ktlo=ktlo, kthi=kthi, col=col))
            col += kthi - ktlo
            q0 += qlen
    return chunks, col


def _build(chunks, ncols):
    nc = bacc.Bacc(None, target_bir_lowering=False, debug=False)

    hsT = nc.declare_dram_parameter("hsT", [HIDDEN, L], BF, isOutput=False)
    wqT = nc.declare_dram_parameter("wqT", [HIDDEN, FS], BF, isOutput=False)
    wkT = nc.declare_dram_parameter("wkT", [HIDDEN, FS], BF, isOutput=False)
    wvT = nc.declare_dram_parameter("wvT", [HIDDEN, FS], BF, isOutput=False)
    woT = nc.declare_dram_parameter("woT", [NCORES * FS, FS], BF, isOutput=False)
    cosT = nc.declare_dram_parameter("cosT", [HD, L], BF, isOutput=False)
    sinT = nc.declare_dram_parameter("sinT", [HD, L], BF, isOutput=False)
    biasd = nc.declare_dram_parameter("biasd", [HD, ncols], F32, isOutput=False)
    out_ext = nc.declare_dram_parameter("out", [L, FS], F32, isOutput=True)

    HL = L // 2
    ag_in0 = nc.dram_tensor("ag_in0", [FS, HL], BF)
    ag_in1 = nc.dram_tensor("ag_in1", [FS, HL], BF)
    ag_out0 = nc.dram_tensor("ag_out0", [NCORES * FS, HL], BF, addr_space="Shared")
    ag_out1 = nc.dram_tensor("ag_out1", [NCORES * FS, HL], BF, addr_space="Shared")

    KC = HIDDEN // 128  # 8 contraction chunks

    with tile.TileContext(nc) as tc:
        with tc.tile_pool(name="avnp", bufs=1) as pv:
            avn0 = pv.tile([128, HPC, HL], BF)
            avn1 = pv.tile([128, HPC, HL], BF)
            with tc.tile_pool(name="persist", bufs=1) as pa:
                qT_sb = pa.tile([128, HPC, L], BF)
                kT_sb = pa.tile([128, HPC, L], BF)
                v_sb = pa.tile([128, L // 128, FS], BF)
                bias_sb = pa.tile([128, ncols], F32)
                ones_col = pa.tile([128, 1], BF)
                ones_row = pa.tile([1, 128], F32)

                nc.vector.memset(ones_col[:], 1.0)
                nc.vector.memset(ones_row[:], 1.0)
                nc.sync.dma_start(bias_sb[:], biasd[:, :])

                # ---------------- phase 1: projections + rope ----------------
                with (
                    tc.tile_pool(name="p1", bufs=1) as p1,
                    tc.tile_pool(name="w1", bufs=4) as w1,
                    tc.tile_pool(name="ps1", bufs=2, space="PSUM") as ps1,
                ):
                    hsT_sb = p1.tile([128, KC, L], BF)
                    wq_sb = p1.tile([128, KC, FS], BF)
                    wk_sb = p1.tile([128, KC, FS], BF)
                    wv_sb = p1.tile([128, KC, FS], BF)
                    cos_sb = p1.tile([128, L], BF)
                    sin_sb = p1.tile([128, L], BF)

                    for kc in range(KC):
                        nc.sync.dma_start(hsT_sb[:, kc, :], hsT[ts(kc, 128), :])
                        nc.sync.dma_start(wq_sb[:, kc, :], wqT[ts(kc, 128), :])
                        nc.sync.dma_start(wk_sb[:, kc, :], wkT[ts(kc, 128), :])
                        nc.sync.dma_start(wv_sb[:, kc, :], wvT[ts(kc, 128), :])
                    nc.sync.dma_start(cos_sb[:], cosT[:, :])
                    nc.sync.dma_start(sin_sb[:], sinT[:, :])

                    # q/k projections per head in (d x tokens) layout + rope
                    for h in range(HPC):
                        for tt in range(L // 512):
                            for w_sb, dst in ((wq_sb, qT_sb), (wk_sb, kT_sb)):
                                psum = ps1.tile([128, 512], F32, tag="proj")
                                for kc in range(KC):
                                    nc.tensor.matmul(
                                        psum[:],
                                        lhsT=w_sb[:, kc, ts(h, 128)],
                                        rhs=hsT_sb[:, kc, ts(tt, 512)],
                                        start=(kc == 0),
                                        stop=(kc == KC - 1),
                                    )
                                raw = w1.tile([128, 512], BF, tag="raw")
                                nc.scalar.copy(raw[:], psum[:])
                                swp = w1.tile([128, 512], BF, tag="swp")
                                nc.sync.dma_start(swp[0:64, :], raw[64:128, :])
                                nc.sync.dma_start(swp[64:128, :], raw[0:64, :])
                                t1 = w1.tile([128, 512], BF, tag="t1")
                                t2 = w1.tile([128, 512], BF, tag="t2")
                                nc.vector.tensor_tensor(
                                    t1[:], raw[:], cos_sb[:, ts(tt, 512)],
                                    op=mybir.AluOpType.mult)
                                nc.vector.tensor_tensor(
                                    t2[:], swp[:], sin_sb[:, ts(tt, 512)],
                                    op=mybir.AluOpType.mult)
                                nc.vector.tensor_tensor(
                                    dst[:, h, ts(tt, 512)], t1[:], t2[:],
                                    op=mybir.AluOpType.add)

                    # v projection in (tokens x d) layout
                    for tt in range(L // 128):
                        psum = ps1.tile([128, FS], F32, tag="proj")
                        for kc in range(KC):
                            nc.tensor.matmul(
                                psum[:, :FS],
                                lhsT=hsT_sb[:, kc, ts(tt, 128)],
                                rhs=wv_sb[:, kc, :],
                                start=(kc == 0),
                                stop=(kc == KC - 1),
                            )
                        nc.scalar.copy(v_sb[:, tt, :], psum[:, :FS])

                # ---------------- phase 2: attention + halved all-gather ----
                last_h0 = max(i for i, c in enumerate(chunks)
                              if c["q0"] + c["qlen"] <= HL)

                def emit_ag(half, avn_h, ag_in_h, ag_out_h):
                    for h in range(HPC):
                        nc.sync.dma_start(ag_in_h[ts(h, 128), :], avn_h[:, h, :])
                    nc.gpsimd.collective_compute(
                        "AllGather",
                        mybir.AluOpType.bypass,
                        replica_groups=[list(range(NCORES))],
                        ins=[ag_in_h.ap().opt()],
                        outs=[ag_out_h.ap().opt()],
                    )

                with (
                    tc.tile_pool(name="w2", bufs=4) as w2,
                    tc.tile_pool(name="ps2", bufs=2, space="PSUM") as ps2,
                ):
                    for ci, ch in enumerate(chunks):
                        q0, qlen = ch["q0"], ch["qlen"]
                        ktlo, kthi = ch["ktlo"], ch["kthi"]
                        nkt = kthi - ktlo
                        avn_h, qq = (avn0, q0) if q0 < HL else (avn1, q0 - HL)
                        for h in range(HPC):
                            av = ps2.tile([128, 512], F32, tag="av")
                            rs = ps2.tile([1, 512], F32, tag="rs")
                            for i, kt in enumerate(range(ktlo, kthi)):
                                sc = ps2.tile([128, 512], F32, tag="sc")
                                nc.tensor.matmul(
                                    sc[:, :qlen],
                                    lhsT=kT_sb[:, h, ts(kt, 128)],
                                    rhs=qT_sb[:, h, q0:q0 + qlen],
                                    start=True, stop=True,
                                )
                                ex = w2.tile([128, 512], BF, tag="ex")
                                nc.scalar.activation(
                                    ex[:, :qlen], sc[:, :qlen],
                                    mybir.ActivationFunctionType.Exp,
                                    bias=bias_sb[:, ch["col"] + i:ch["col"] + i + 1],
                                    scale=SCALE,
                                )
                                nc.tensor.matmul(
                                    av[:, :qlen],
                                    lhsT=v_sb[:, kt, ts(h, 128)],
                                    rhs=ex[:, :qlen],
                                    start=(i == 0), stop=(i == nkt - 1),
                                )
                                nc.tensor.matmul(
                                    rs[:1, :qlen],
                                    lhsT=ones_col[:],
                                    rhs=ex[:, :qlen],
                                    start=(i == 0), stop=(i == nkt - 1),
                                )
                            rec = w2.tile([1, 512], F32, tag="rec")
                            nc.vector.reciprocal(rec[:1, :qlen], rs[:1, :qlen])
                            bc = ps2.tile([128, 512], F32, tag="bc")
                            nc.tensor.matmul(
                                bc[:, :qlen], lhsT=ones_row[:], rhs=rec[:1, :qlen],
                                start=True, stop=True,
                            )
                            bcs = w2.tile([128, 512], F32, tag="bcs")
                            nc.vector.tensor_copy(bcs[:, :qlen], bc[:, :qlen])
                            nc.vector.tensor_tensor(
                                avn_h[:, h, qq:qq + qlen],
                                av[:, :qlen], bcs[:, :qlen],
                                op=mybir.AluOpType.mult)
                        if ci == last_h0:
                            emit_ag(0, avn0, ag_in0, ag_out0)
                    emit_ag(1, avn1, ag_in1, ag_out1)

            # ---------------- phase 3: output projection ----------------
            with (
                tc.tile_pool(name="p3", bufs=2) as p3,
                tc.tile_pool(name="pw", bufs=1) as pw,
                tc.tile_pool(name="w3", bufs=4) as w3,
                tc.tile_pool(name="ps3", bufs=2, space="PSUM") as ps3,
            ):
                wo_sb = pw.tile([128, 2 * KC, FS], BF)
                for fc in range(2 * KC):
                    nc.sync.dma_start(wo_sb[:, fc, :], woT[ts(fc, 128), :])
                for half, ag_out_h in ((0, ag_out0), (1, ag_out1)):
                    attnT = p3.tile([128, 2 * KC, HL], BF, tag="attn")
                    for fc in range(2 * KC):
                        nc.sync.dma_start(attnT[:, fc, :], ag_out_h[ts(fc, 128), :])
                    for tt in range(HL // 128):
                        wp = ps3.tile([128, FS], F32, tag="wo")
                        for fc in range(2 * KC):
                            nc.tensor.matmul(
                                wp[:],
                                lhsT=attnT[:, fc, ts(tt, 128)],
                                rhs=wo_sb[:, fc, :],
                                start=(fc == 0), stop=(fc == 2 * KC - 1),
                            )
                        ot = w3.tile([128, FS], F32, tag="ot")
                        nc.vector.tensor_copy(ot[:], wp[:])
                        nc.sync.dma_start(
                            out_ext[ts(half * (HL // 128) + tt, 128), :], ot[:])

    return nc


def _host_inputs(hidden_states, wq, wk, wv, wo, attention_mask, segment_ids,
                 position_ids, chunks, ncols):
    perm = np.concatenate([np.arange(0, HD, 2), np.arange(1, HD, 2)])
    inv = (1.0 / (THETA ** (np.arange(0, HD, 2, dtype=np.float64) / HD))).astype(
        np.float64)

    in_maps = []
    for c in range(NCORES):
        b, hp = c // TP, c % TP
        # zero-padded extended wo: only the rows of my batch's ranks are live,
        # so the SPMD-uniform 16-chunk wo matmul picks out my batch's features
        woT = np.zeros((NCORES * FS, FS), dtype=BF16)
        for r in range(NCORES):
            if r // TP == b:
                woT[r * FS:(r + 1) * FS, :] = (
                    wo[hp * FS:(hp + 1) * FS,
                       (r % TP) * FS:(r % TP + 1) * FS].T).astype(BF16)
        hsT = np.ascontiguousarray(hidden_states[b].T).astype(BF16)

        def wshard(w, permute):
            cols = []
            for h in range(HPC):
                gh = HPC * hp + h
                rows = w[gh * HD:(gh + 1) * HD]
                if permute:
                    rows = rows[perm]
                cols.append(rows.T)  # (1024, 128)
            return np.ascontiguousarray(np.concatenate(cols, axis=1)).astype(BF16)

        ang = position_ids[b].astype(np.float64)[None, :] * inv[:, None]  # (64,L)
        cs = np.cos(ang)
        sn = np.sin(ang)
        cosT = np.concatenate([cs, cs], axis=0).astype(BF16)
        sinT = np.concatenate([-sn, sn], axis=0).astype(BF16)

        biasd = np.zeros((HD, ncols), dtype=np.float32)
        amask = attention_mask[b].astype(bool)
        keyidx = np.arange(HD)
        for ch in chunks:
            lo, hi = ch["per_b"][b]
            for i, kt in enumerate(range(ch["ktlo"], ch["kthi"])):
                keys = kt * HD + keyidx
                ok = (keys >= lo) & (keys < hi) & amask[np.clip(keys, 0, L - 1)]
                biasd[:, ch["col"] + i] = np.where(ok, 0.0, NEG)

        in_maps.append({
            "hsT": hsT,
            "wqT": wshard(wq, True),
            "wkT": wshard(wk, True),
            "wvT": wshard(wv, False),
            "woT": woT,
            "cosT": cosT,
            "sinT": sinT,
            "biasd": biasd,
        })
    return in_maps


def kernel(hidden_states, wq, wk, wv, wo, attention_mask, segment_ids,
           position_ids):
    global LAST_RESULTS
    hidden_states = np.asarray(hidden_states, dtype=np.float32)
    wq = np.asarray(wq, dtype=np.float32)
    wk = np.asarray(wk, dtype=np.float32)
    wv = np.asarray(wv, dtype=np.float32)
    wo = np.asarray(wo, dtype=np.float32)
    attention_mask = np.asarray(attention_mask)
    segment_ids = np.asarray(segment_ids)
    position_ids = np.asarray(position_ids)

    seg_starts = [int(np.searchsorted(segment_ids[b], 1)) for b in range(B)]
    chunks, ncols = _plan_chunks(seg_starts)

    nc = _build(chunks, ncols)
    nc.finalize()
    in_maps = _host_inputs(hidden_states, wq, wk, wv, wo, attention_mask,
                           segment_ids, position_ids, chunks, ncols)

    prof_dir = os.environ.get("KPROF_DIR")
    if prof_dir:
        import contextlib
        import ctypes

        lib = ctypes.CDLL("/opt/axon/libaxon_pjrt.so")
        lib.axon_start_nrt_profile.argtypes = [
            ctypes.POINTER(ctypes.c_int64), ctypes.c_size_t]
        lib.axon_start_nrt_profile.restype = ctypes.c_int64
        lib.axon_stop_nrt_profile.argtypes = [ctypes.c_char_p]
        lib.axon_stop_nrt_profile.restype = ctypes.c_int64

        @contextlib.contextmanager
        def _hook(output_dir, device_ids):
            import jax
            jax.devices()
            if device_ids:
                ids = (ctypes.c_int64 * len(device_ids))(*device_ids)
                rc = lib.axon_start_nrt_profile(ids, len(device_ids))
            else:
                rc = lib.axon_start_nrt_profile(None, 0)
            if rc != 0:
                raise RuntimeError(f"axon_start_nrt_profile rc={rc}")
            try:
                yield
            finally:
                n = lib.axon_stop_nrt_profile(str(output_dir).encode())
                print(f"profile: {n} file(s) written to {output_dir}")

        with _hook(prof_dir, list(range(NCORES))):
            res = run_bass_kernel_spmd(nc, in_maps, core_ids=list(range(NCORES)))
    else:
        res = run_bass_kernel_spmd(nc, in_maps, core_ids=list(range(NCORES)))
    LAST_RESULTS = res

    out = np.zeros((B, L, HIDDEN), dtype=np.float32)
    for c in range(NCORES):
        b, hp = c // TP, c % TP
        out[b, :, hp * FS:(hp + 1) * FS] = res.results[c]["out"]
    return out
